# revision 5
# baseline (speedup 1.0000x reference)
"""LeNet-style CNN (conv5x5+avgpool2+sigmoid x2, then 3 FC layers) on 8 trn2
NeuronCores, pure data parallel over the batch.

Key ideas:
- conv(5x5, stride 1) followed by 2x2 avg-pool is algebraically a single
  6x6 stride-2 convolution (pooling is linear) -> each conv layer is one
  strided conv: 28x28 -> 12x12 (10ch), 12x12 -> 4x4 (20ch).
- Activations live in SBUF as [feature partitions, (spatial-major, batch)]
  free layout. The strided convs then become accumulating matmuls with
  Toeplitz-structured weights reading *strided views* of the previous
  layer - no im2col data movement on device at all.
- x is pre-transposed on the host to [(row-parity, width)=56 partitions,
  (row-half, batch)] so the device DMA is fully contiguous and the row
  stride-2 of layer 1 turns into partition-parity selection.
- Matmuls run in float32r (FP22 mantissa-13) which is full rate for
  moving-dim >= 256; accumulation is fp32 in PSUM. Bias+sigmoid fuse into
  one ScalarE activation per tile.
"""

import numpy as np
import concourse.bacc as bacc
import concourse.mybir as mybir
import concourse.tile as tile
from concourse.bass_utils import run_bass_kernel_spmd

F32 = mybir.dt.float32
F32R = mybir.dt.float32r
SIG = mybir.ActivationFunctionType.Sigmoid

N_CORES = 8
B_FULL = 8192
NB = B_FULL // N_CORES  # 1024 images per core


def _fuse_pool(W):
    """conv(W, stride 1) + 2x2 mean-pool == conv(Wf, stride 2), Wf 6x6."""
    O, C, _, _ = W.shape
    Wf = np.zeros((O, C, 6, 6), np.float32)
    for u in (0, 1):
        for v in (0, 1):
            Wf[:, :, u : u + 5, v : v + 5] += W
    return Wf * 0.25


def _host_weights(W1, b1, W2, b2, L1, Lb1, L2, Lb2, L3, Lb3):
    W1f = _fuse_pool(W1)  # [10,1,6,6]
    W2f = _fuse_pool(W2)  # [20,10,6,6]

    # Layer 1 Toeplitz: T1[k][(par,w), (pj,o)] = W1f[o,0,2k+par,w-2pj]
    T1 = np.zeros((3, 56, 120), np.float32)
    for k in range(3):
        for par in range(2):
            e = 2 * k + par
            for pj in range(12):
                for f in range(6):
                    w = 2 * pj + f
                    T1[k, par * 28 + w, pj * 10 : pj * 10 + 10] = W1f[:, 0, e, f]

    # Layer 2 Toeplitz: T2[par,k][(pj,c), (qj,oc)] = W2f[oc,c,2k+par,pj-2qj]
    T2 = np.zeros((6, 120, 80), np.float32)
    for par in range(2):
        for k in range(3):
            e = 2 * k + par
            for qj in range(4):
                for f in range(6):
                    pj = 2 * qj + f
                    for c in range(10):
                        T2[par * 3 + k, pj * 10 + c, qj * 20 : qj * 20 + 20] = W2f[
                            :, c, e, f
                        ]

    # FC1 permuted for the [(qj,oc) partitions, (qi,b) free] input layout:
    # 4 accumulation steps over qi.
    L1p = np.zeros((4, 80, 120), np.float32)
    for qi in range(4):
        for qj in range(4):
            for oc in range(20):
                L1p[qi, qj * 20 + oc] = L1[oc * 16 + qi * 4 + qj]

    bias1 = np.ascontiguousarray(
        np.tile(np.asarray(b1).reshape(10), 12).reshape(120, 1), dtype=np.float32
    )
    bias2 = np.ascontiguousarray(
        np.tile(np.asarray(b2).reshape(20), 4).reshape(80, 1), dtype=np.float32
    )
    lb1 = np.ascontiguousarray(np.asarray(Lb1).reshape(120, 1), dtype=np.float32)
    lb2 = np.ascontiguousarray(np.asarray(Lb2).reshape(84, 1), dtype=np.float32)
    # FC3 bias folded in via a constant-one activation row.
    L3b = np.ascontiguousarray(
        np.concatenate([np.asarray(L3), np.asarray(Lb3).reshape(1, 10)], axis=0),
        dtype=np.float32,
    )  # [85, 10]
    return {
        "t1": T1,
        "t2": T2,
        "l1p": L1p,
        "l2w": np.ascontiguousarray(L2, dtype=np.float32),  # [120,84]
        "l3b": L3b,
        "bias1": bias1,
        "bias2": bias2,
        "lb1": lb1,
        "lb2": lb2,
    }


def _build_nc():
    nc = bacc.Bacc()
    xp = nc.dram_tensor("xp", [56, 14 * NB], F32R, kind="ExternalInput")
    t1 = nc.dram_tensor("t1", [3, 56, 120], F32R, kind="ExternalInput")
    t2 = nc.dram_tensor("t2", [6, 120, 80], F32R, kind="ExternalInput")
    l1p = nc.dram_tensor("l1p", [4, 80, 120], F32R, kind="ExternalInput")
    l2w = nc.dram_tensor("l2w", [120, 84], F32R, kind="ExternalInput")
    l3b = nc.dram_tensor("l3b", [85, 10], F32, kind="ExternalInput")
    bias1 = nc.dram_tensor("bias1", [120, 1], F32, kind="ExternalInput")
    bias2 = nc.dram_tensor("bias2", [80, 1], F32, kind="ExternalInput")
    lb1 = nc.dram_tensor("lb1", [120, 1], F32, kind="ExternalInput")
    lb2 = nc.dram_tensor("lb2", [84, 1], F32, kind="ExternalInput")
    y = nc.dram_tensor("y", [NB, 10], F32, kind="ExternalOutput")

    with tile.TileContext(nc) as tc:
        with (
            tc.tile_pool(name="w", bufs=1) as wp,
            tc.tile_pool(name="act", bufs=1) as ap,
            tc.tile_pool(name="ps", bufs=6, space="PSUM") as psp,
            tc.tile_pool(name="psy", bufs=2, space="PSUM") as psyp,
        ):
            # --- weights/constants to SBUF ---
            t1s = []
            for k in range(3):
                t = wp.tile([56, 120], F32R, tag=f"t1_{k}")
                nc.sync.dma_start(t[:], t1[k])
                t1s.append(t)
            t2s = []
            for i in range(6):
                t = wp.tile([120, 80], F32R, tag=f"t2_{i}")
                nc.sync.dma_start(t[:], t2[i])
                t2s.append(t)
            l1s = []
            for qi in range(4):
                t = wp.tile([80, 120], F32R, tag=f"l1p_{qi}")
                nc.sync.dma_start(t[:], l1p[qi])
                l1s.append(t)
            l2s = wp.tile([120, 84], F32R, tag="l2w")
            nc.sync.dma_start(l2s[:], l2w[:])
            l3s = wp.tile([85, 10], F32, tag="l3b")
            nc.sync.dma_start(l3s[:], l3b[:])
            b1s = wp.tile([120, 1], F32, tag="bias1")
            nc.sync.dma_start(b1s[:], bias1[:])
            b2s = wp.tile([80, 1], F32, tag="bias2")
            nc.sync.dma_start(b2s[:], bias2[:])
            lb1s = wp.tile([120, 1], F32, tag="lb1")
            nc.sync.dma_start(lb1s[:], lb1[:])
            lb2s = wp.tile([84, 1], F32, tag="lb2")
            nc.sync.dma_start(lb2s[:], lb2[:])

            # --- input: [56=(par,w), (rh, b)] - contiguous chunked load ---
            xs = ap.tile([56, 14 * NB], F32R, tag="xp")
            for c in range(7):
                sl = slice(c * 2 * NB, (c + 1) * 2 * NB)
                nc.sync.dma_start(xs[:, sl], xp[:, sl])

            # --- activations ---
            # h1: [(pj,o)=120, free = par*6*NB + pih*NB + b], pi = 2*pih+par
            h1 = ap.tile([120, 12 * NB], F32R, tag="h1")
            # h2: [(qj,oc)=80, free = qi*NB + b]
            h2 = ap.tile([80, 4 * NB], F32R, tag="h2")
            h3 = ap.tile([120, NB], F32R, tag="h3")
            h4 = ap.tile([85, NB], F32, tag="h4")  # row 84 == 1.0 (FC3 bias)
            ys = ap.tile([128, 80], F32, tag="ys")

            # Row 84 must be 1.0 (FC3 bias row); FC2's activation later
            # overwrites rows 0..83, so filling the whole tile is fine and
            # keeps the memset base-partition at 0 (gpsimd alignment rule).
            nc.gpsimd.memset(h4[:, :], 1.0)

            HB = 512  # batch-half tile
            # --- layer 1: 12 pi x 2 halves, 3 accumulating matmuls each ---
            for pi in range(12):
                hoff = ((pi % 2) * 6 + pi // 2) * NB
                for h in range(2):
                    b0 = h * HB
                    ps = psp.tile([120, HB], F32, tag="ps")
                    for k in range(3):
                        rhs = xs[:, (pi + k) * NB + b0 : (pi + k) * NB + b0 + HB]
                        nc.tensor.matmul(
                            ps[:],
                            t1s[k][:],
                            rhs,
                            start=(k == 0),
                            stop=(k == 2),
                        )
                    nc.scalar.activation(
                        h1[:, hoff + b0 : hoff + b0 + HB], ps[:], SIG, bias=b1s[:]
                    )

            # --- layer 2: 4 qi x 2 halves, 6 accumulating matmuls each ---
            for qi in range(4):
                for h in range(2):
                    b0 = h * HB
                    ps = psp.tile([80, HB], F32, tag="ps")
                    n = 0
                    for par in range(2):
                        for k in range(3):
                            off = (par * 6 + (qi + k)) * NB + b0
                            nc.tensor.matmul(
                                ps[:],
                                t2s[par * 3 + k][:],
                                h1[:, off : off + HB],
                                start=(n == 0),
                                stop=(n == 5),
                            )
                            n += 1
                    nc.scalar.activation(
                        h2[:, qi * NB + b0 : qi * NB + b0 + HB], ps[:], SIG, bias=b2s[:]
                    )

            # --- FC1 (320->120): 4 accumulating matmuls over qi ---
            for h in range(2):
                b0 = h * HB
                ps = psp.tile([120, HB], F32, tag="ps")
                for qi in range(4):
                    nc.tensor.matmul(
                        ps[:],
                        l1s[qi][:],
                        h2[:, qi * NB + b0 : qi * NB + b0 + HB],
                        start=(qi == 0),
                        stop=(qi == 3),
                    )
                nc.scalar.activation(h3[:, b0 : b0 + HB], ps[:], SIG, bias=lb1s[:])

            # --- FC2 (120->84) ---
            for h in range(2):
                b0 = h * HB
                ps = psp.tile([84, HB], F32, tag="ps")
                nc.tensor.matmul(
                    ps[:],
                    l2s[:],
                    h3[:, b0 : b0 + HB],
                    start=True,
                    stop=True,
                )
                nc.scalar.activation(h4[0:84, b0 : b0 + HB], ps[:], SIG, bias=lb2s[:])

            # --- FC3 (84->10, bias via ones row): activations stationary ---
            for t4 in range(8):
                ps = psyp.tile([128, 10], F32, tag="psy")
                nc.tensor.matmul(
                    ps[:],
                    h4[:, t4 * 128 : (t4 + 1) * 128],
                    l3s[:],
                    start=True,
                    stop=True,
                )
                nc.vector.tensor_copy(ys[:, t4 * 10 : (t4 + 1) * 10], ps[:])

            # --- output: y[t4*128+p, n] = ys[p, t4*10+n] ---
            for t4 in range(8):
                nc.sync.dma_start(
                    y[t4 * 128 : (t4 + 1) * 128, :], ys[:, t4 * 10 : (t4 + 1) * 10]
                )
    nc.compile()
    return nc


_NC_CACHE = None


def _get_nc():
    global _NC_CACHE
    if _NC_CACHE is None:
        _NC_CACHE = _build_nc()
    return _NC_CACHE


def _make_in_maps(x, W1, b1, W2, b2, L1, Lb1, L2, Lb2, L3, Lb3):
    wmap = _host_weights(W1, b1, W2, b2, L1, Lb1, L2, Lb2, L3, Lb3)
    x = np.asarray(x, dtype=np.float32)
    in_maps = []
    for c in range(N_CORES):
        xc = x[c * NB : (c + 1) * NB, 0]  # [NB, 28, 28]
        xpc = np.ascontiguousarray(
            xc.reshape(NB, 14, 2, 28).transpose(2, 3, 1, 0).reshape(56, 14 * NB)
        )
        m = {"xp": xpc}
        m.update(wmap)
        in_maps.append(m)
    return in_maps


def _run(trace=False, **inputs):
    nc = _get_nc()
    in_maps = _make_in_maps(**inputs)
    res = run_bass_kernel_spmd(nc, in_maps, list(range(N_CORES)), trace=trace)
    out = np.concatenate([res.results[i]["y"] for i in range(N_CORES)], axis=0)
    return out, res


def kernel(**inputs):
    out, _ = _run(trace=False, **inputs)
    return out
